# revision 51
# baseline (speedup 1.0000x reference)
"""CayleyConvED Trainium2 kernel (8-core SPMD, frequency-sharded), v2.

Math (matches reference.py):
  xfft = rfft2(x)                         -> per-freq [cin, B] complex
  W[f,i,j] = sum_t w[i,j,t] * exp(+2pi*i*(u*(p-1)+v*(q-1))/32)   (t=(p,q))
  A = c*(W - W^H),  c = alpha/||W||_F     (skew-Hermitian, ||A||_2 ~ 0.04)
  Q = (I+A)^{-1}(I-A) = I - 2A + 2A^2 - ...
  v = Q@x ~= x - 2Ax  (first-order Neumann; rel err ~ 2||A||^2 ~ 1e-3)
  cwx = H v;  y = irfft2(cwx) + bias

v2 vs v1: one Neumann step instead of 3 (tolerance is 2e-2); all heavy
matmuls in bf16 (fp32 matmul issues 2 HW passes); A / H / x / idft / the
AllToAll payload staged in bf16 (halves HBM+collective traffic); the -2
factor of (I - 2A) is folded into the A planes at build time.

FFTs are dense matmuls against host-precomputed DFT basis matrices. wnorm
uses the closed form ||W||^2 = 32*(16*||w||^2 + ||sum_q w||^2)  (fp32).

Sharding: F=544 freqs = 8 cores x 68. The final inverse-FFT contraction over
all freqs uses an on-device AllToAll (each core keeps cout slice
[32c, 32c+32)), then per-core dense matmul; host concatenates cout slices.
"""

import numpy as np
import ml_dtypes

import concourse.bass as bass
import concourse.mybir as mybir
import concourse.tile as tile
from concourse import bacc
from concourse import bass_utils

N_CORES = 8
B, C, N = 16, 256, 32
NF = N // 2 + 1          # 17
F = N * NF               # 544
FL = F // N_CORES        # 68 freqs per core
F2L = 2 * FL             # 136 re/im rows per core
PIX = N * N              # 1024
FP32 = mybir.dt.float32
BF16 = mybir.dt.bfloat16
NPBF = ml_dtypes.bfloat16
ALU = mybir.AluOpType
ACTF = mybir.ActivationFunctionType


# ----------------------------------------------------------------------------
# Host-side constants (input-independent)
# ----------------------------------------------------------------------------

def _host_consts():
    # forward: rfft2 response of each pixel impulse; f = u*NF + v
    imp = np.zeros((PIX, N, N))
    imp[np.arange(PIX), np.arange(PIX) // N, np.arange(PIX) % N] = 1.0
    dft_c = np.fft.rfft2(imp).reshape(PIX, F)          # [pix, f] complex

    # inverse: irfft2 response of unit re/im at each bin -> [2F, PIX]
    basis = np.zeros((2 * F, N, NF), np.complex128)
    fidx = np.arange(F)
    basis[2 * fidx, fidx // NF, fidx % NF] = 1.0
    basis[2 * fidx + 1, fidx // NF, fidx % NF] = 1.0j
    idft = np.fft.irfft2(basis, s=(N, N)).reshape(2 * F, PIX)

    # phase basis: W[f,i,j] = sum_t w[i,j,t] ph[f,t], t = p*3+q
    u = (np.arange(F) // NF).reshape(F, 1)
    v = (np.arange(F) % NF).reshape(F, 1)
    p = (np.arange(9) // 3).reshape(1, 9)
    q = (np.arange(9) % 3).reshape(1, 9)
    ph = np.exp(2j * np.pi * (u * (p - 1) + v * (q - 1)) / N)  # [F, 9]
    return dft_c, idft, ph


# ----------------------------------------------------------------------------
# Device program (SPMD: one NEFF, per-core data via in_maps)
# ----------------------------------------------------------------------------

def _build_nc():
    nc = bacc.Bacc("TRN2", target_bir_lowering=False, debug=False,
                   enable_asserts=True, num_devices=N_CORES)

    def din(name, shape, dt=FP32):
        return nc.dram_tensor(name, shape, dt, kind="ExternalInput").ap()

    xt_in = din("xt_in", [PIX, B * C], BF16)
    wflat = din("wflat", [128, 4608])
    wmT = din("wmT", [9, 65536], BF16)
    wpT = din("wpT", [9, 65536], BF16)
    alpha_in = din("alpha_in", [1])
    h2_in = din("h2_in", [FL, 128, 2, 2, C], BF16)  # [f, p, {re,im}, jc, i]
    dft_in = din("dft_in", [PIX, F2L], BF16)
    # rows h*640+0..543 = idft half h (perm640), 1280 = ones, rest zero
    idft_in = din("idft_in", [11 * 128, PIX], BF16)
    phre_in = din("phre_in", [9, FL])
    nphim_in = din("nphim_in", [9, FL])
    bias_exp = din("bias_exp", [512], BF16)    # repeat(bias[co-slice], 16)
    y_out = nc.dram_tensor("y_out", [32, B, PIX], FP32, kind="ExternalOutput").ap()

    with tile.TileContext(nc) as tc:
        with tc.tile_pool(name="const", bufs=1) as pc, \
             tc.tile_pool(name="dram", bufs=1, space="DRAM") as pdram:
            dft_sb = pc.tile([128, 8, F2L], BF16)
            nc.sync.dma_start(dft_sb[:], dft_in.rearrange("(k p) f -> p k f", p=128))
            idft_sb = pc.tile([128, 11, PIX], BF16)
            nc.sync.dma_start(idft_sb[:], idft_in.rearrange("(k p) f -> p k f", p=128))
            ones9 = pc.tile([128, 9], FP32)
            nc.vector.memset(ones9[:], 1.0)
            phre_b = pc.tile([9, FL], BF16)    # scaled by -2c
            nphim_b = pc.tile([9, FL], BF16)
            # xfft staging: [cin%128, cchunk, f2(local), b]  fp32 + bf16
            xfft_f = pc.tile([128, 2, F2L, 16], FP32)
            xfft_b = pc.tile([128, 2, F2L, 16], BF16)

            # [f, p, jc, plane, i]: reader tile [128p, jc, plane, i] is fully
            # contiguous per partition (2KB runs)
            abuf = pdram.tile([FL, 128, 2, 2, 256], BF16)   # -2*A_re / +2*A_im
            # two half-tensors so each AllToAll is a full-tensor op; half 0's
            # exchange overlaps the second half of the main loop
            cwx_l = [pdram.tile([N_CORES, FL, 32, 16], BF16, name=f"cwx_l{h}")
                     for h in range(2)]
            cwx_g = [pdram.tile([N_CORES, FL, 32, 16], BF16, name=f"cwx_g{h}")
                     for h in range(2)]

            # ---------------- wnorm -> c, scale phase tables ----------------
            with tc.tile_pool(name="wn", bufs=1) as pw, \
                 tc.tile_pool(name="wnp", bufs=1, space="PSUM") as pwp:
                w_sb = pw.tile([128, 512, 3, 3], FP32)
                nc.sync.dma_start(w_sb[:], wflat.rearrange("p (c u v) -> p c u v", u=3, v=3))
                w_fl = w_sb.rearrange("p a u v -> p (a u v)")
                sq = pw.tile([128, 4608], FP32)
                nc.vector.tensor_tensor(sq[:], w_fl, w_fl, ALU.mult)
                acc1 = pw.tile([128, 1], FP32)
                nc.vector.tensor_reduce(acc1[:], sq[:], mybir.AxisListType.X, ALU.add)
                s3 = pw.tile([128, 512, 3], FP32)
                nc.vector.tensor_tensor(s3[:], w_sb[:, :, :, 0], w_sb[:, :, :, 1], ALU.add)
                nc.vector.tensor_tensor(s3[:], s3[:], w_sb[:, :, :, 2], ALU.add)
                sq2 = pw.tile([128, 512, 3], FP32)
                nc.vector.tensor_tensor(sq2[:], s3[:], s3[:], ALU.mult)
                acc2 = pw.tile([128, 1], FP32)
                nc.vector.tensor_reduce(acc2[:], sq2.rearrange("p a b -> p (a b)"),
                                        mybir.AxisListType.X, ALU.add)
                tot = pw.tile([128, 1], FP32)
                nc.vector.tensor_scalar(tot[:], acc1[:], 16.0, None, ALU.mult)
                nc.vector.tensor_tensor(tot[:], tot[:], acc2[:], ALU.add)
                s9p = pwp.tile([9, 1], FP32)
                nc.tensor.matmul(s9p[:], ones9[:, 0:9], tot[:], start=True, stop=True)
                s9 = pw.tile([9, 1], FP32)
                nc.vector.tensor_copy(s9[:], s9p[:])
                rt9 = pw.tile([9, 1], FP32)
                nc.scalar.activation(rt9[:], s9[:], ACTF.Sqrt, bias=0.0, scale=32.0)
                rc9 = pw.tile([9, 1], FP32)
                nc.vector.reciprocal(rc9[:], rt9[:])
                al9 = pw.tile([9, 1], FP32)
                for t in range(9):
                    nc.sync.dma_start(al9[t:t + 1, 0:1], alpha_in[0:1])
                c9 = pw.tile([9, 1], FP32)
                nc.vector.tensor_tensor(c9[:], rc9[:], al9[:], ALU.mult)
                nc.vector.tensor_scalar(c9[:], c9[:], -2.0, None, ALU.mult)
                phre_r = pw.tile([9, FL], FP32)
                nphim_r = pw.tile([9, FL], FP32)
                nc.sync.dma_start(phre_r[:], phre_in[:])
                nc.sync.dma_start(nphim_r[:], nphim_in[:])
                phre_s = pw.tile([9, FL], FP32)
                nphim_s = pw.tile([9, FL], FP32)
                nc.vector.tensor_scalar(phre_s[:], phre_r[:], c9[:, 0:1], None, ALU.mult)
                nc.vector.tensor_scalar(nphim_s[:], nphim_r[:], c9[:, 0:1], None, ALU.mult)
                nc.vector.tensor_copy(phre_b[:], phre_s[:])
                nc.vector.tensor_copy(nphim_b[:], nphim_s[:])

            # --- A build: abuf[plane, f, j*256+i] = {-2A_re, +2A_im}[i,j] ---
            with tc.tile_pool(name="ab", bufs=2) as pa, \
                 tc.tile_pool(name="abp", bufs=4, space="PSUM") as pap:
                for e8 in range(8):
                    wm8 = pa.tile([9, 8192], BF16, tag="wm8")
                    wp8 = pa.tile([9, 8192], BF16, tag="wp8")
                    nc.sync.dma_start(wm8[:], wmT[:, e8 * 8192:(e8 + 1) * 8192])
                    nc.scalar.dma_start(wp8[:], wpT[:, e8 * 8192:(e8 + 1) * 8192])
                    sbA8 = pa.tile([FL, 2, 8192], BF16, tag="sbA8")
                    for c2 in range(16):
                        for plane, (ph_t, rhs_t) in enumerate(
                                ((phre_b, wm8), (nphim_b, wp8))):
                            psA = pap.tile([FL, 512], FP32, tag="psA")
                            nc.tensor.matmul(psA[:], ph_t[:],
                                             rhs_t[:, c2 * 512:(c2 + 1) * 512],
                                             start=True, stop=True)
                            if (c2 + plane) % 2 == 0:
                                nc.vector.tensor_copy(
                                    sbA8[:, plane, c2 * 512:(c2 + 1) * 512], psA[:])
                            else:
                                nc.scalar.activation(
                                    sbA8[:, plane, c2 * 512:(c2 + 1) * 512],
                                    psA[:], ACTF.Copy)
                    for plane in range(2):
                        (nc.sync if (e8 + plane) % 2 else nc.scalar).dma_start(
                            abuf[:, (e8 % 4) * 32:(e8 % 4 + 1) * 32,
                                 e8 // 4, plane, :],
                            sbA8[:, plane, :].rearrange("f (j i) -> f j i", i=256))

            # ---------------- forward FFT of x ----------------
            xt_v = xt_in.rearrange("(k p) c -> p k c", p=128)
            with tc.tile_pool(name="xf", bufs=4) as px, \
                 tc.tile_pool(name="xfo", bufs=2, space="PSUM") as pxo:
                for b in range(B):
                    xall = px.tile([128, 8, 256], BF16, tag="xall")
                    (nc.scalar if b % 2 else nc.sync).dma_start(
                        xall[:], xt_v[:, :, b * C:(b + 1) * C])
                    for cc in range(2):
                        psX = pxo.tile([128, F2L], FP32, tag="psX")
                        for k in range(8):
                            nc.tensor.matmul(psX[:],
                                             xall[:, k, cc * 128:cc * 128 + 128],
                                             dft_sb[:, k, :],
                                             start=(k == 0), stop=(k == 7))
                        nc.vector.tensor_copy(xfft_f[:, cc, :, b], psX[:])
                        nc.scalar.activation(xfft_b[:, cc, :, b], psX[:], ACTF.Copy)

            # ---------------- per-frequency main loop ----------------
            # pa = (-2A_re) @ [xre,xim], pb = (+2A_im) @ [xre,xim]
            # v_re = xre + pa0 + pb1 ; v_im = xim + pa1 - pb0   (v = x - 2Ax)
            # qa = Hre @ v, qb = Him @ v ; cw = (qa0-qb1, qa1+qb0)
            # cw accumulates in SBUF [p, cc, f, e, b]; flushed to DRAM in
            # per-dest-core chunks per f-half, each half's AllToAll overlapping
            # the second half of the main loop.
            cwsb = pc.tile([128, 2, FL, 2, 16], BF16)

            def flush_dma(hs, d):
                src = cwsb[(d % 4) * 32:(d % 4 + 1) * 32, d // 4,
                           hs * 34:(hs + 1) * 34].rearrange("p f e b -> p (f e) b")
                dst = cwx_l[hs][d].rearrange("f2 c b -> c f2 b")
                (nc.scalar if d % 2 else nc.sync).dma_start(dst, src)

            def collective(hs):
                nc.gpsimd.collective_compute(
                    "AllToAll", ALU.bypass,
                    replica_groups=[list(range(N_CORES))],
                    ins=[cwx_l[hs].opt()], outs=[cwx_g[hs].opt()],
                )

            with tc.tile_pool(name="mn", bufs=4) as pm, \
                 tc.tile_pool(name="mz", bufs=4) as pz, \
                 tc.tile_pool(name="mp", bufs=2, space="PSUM") as pmp, \
                 tc.tile_pool(name="mq", bufs=2, space="PSUM") as pmq:
                for g in range(FL // 2):
                    a8 = pm.tile([128, 2, 2, 2, 256], BF16, tag="a8")
                    nc.scalar.dma_start(
                        a8[:], abuf.rearrange("f p jc pl i -> p f jc pl i")
                        [:, 2 * g:2 * g + 2])
                    h8 = pm.tile([128, 2, 2, 2, 256], BF16, tag="h8")
                    nc.sync.dma_start(
                        h8[:], h2_in.rearrange("f p k c i -> p f k c i")
                        [:, 2 * g:2 * g + 2])
                    for q in range(2):
                        fl = 2 * g + q
                        a4 = a8[:, q]                              # [p,jc,pl,i]
                        h4 = h8[:, q]                              # [p,k,jc,i]
                        x2b = xfft_b[:, :, 2 * fl:2 * fl + 2, :]   # bf16
                        x2f = xfft_f[:, :, 2 * fl:2 * fl + 2, :]   # fp32
                        pab = pmp.tile([128, 2, 2, 2, 16], FP32, tag="pab")
                        pa_, pb_ = pab[:, 0], pab[:, 1]
                        for ic in range(2):
                            for jc in range(2):
                                nc.tensor.matmul(pa_[:, ic],
                                                 a4[:, jc, 0, ic * 128:(ic + 1) * 128],
                                                 x2b[:, jc], start=(jc == 0),
                                                 stop=(jc == 1))
                        for ic in range(2):
                            for jc in range(2):
                                nc.tensor.matmul(pb_[:, ic],
                                                 a4[:, jc, 1, ic * 128:(ic + 1) * 128],
                                                 x2b[:, jc], start=(jc == 0),
                                                 stop=(jc == 1))
                        tmp = pz.tile([128, 2, 2, 16], FP32, tag="vt")
                        v = pz.tile([128, 2, 2, 16], BF16, tag="v")
                        nc.any.tensor_tensor(tmp[:], x2f, pa_[:], ALU.add)
                        nc.any.tensor_tensor(v[:, :, 0], tmp[:, :, 0],
                                             pb_[:, :, 1], ALU.add)
                        nc.any.tensor_tensor(v[:, :, 1], tmp[:, :, 1],
                                             pb_[:, :, 0], ALU.subtract)
                        qab = pmq.tile([128, 2, 2, 2, 16], FP32, tag="qab")
                        qa, qb = qab[:, 0], qab[:, 1]
                        for ic in range(2):
                            for jc in range(2):
                                nc.tensor.matmul(qa[:, ic],
                                                 h4[:, 0, jc, ic * 128:(ic + 1) * 128],
                                                 v[:, jc], start=(jc == 0),
                                                 stop=(jc == 1))
                        for ic in range(2):
                            for jc in range(2):
                                nc.tensor.matmul(qb[:, ic],
                                                 h4[:, 1, jc, ic * 128:(ic + 1) * 128],
                                                 v[:, jc], start=(jc == 0),
                                                 stop=(jc == 1))
                        qas = pz.tile([128, 2, 2, 16], FP32, tag="qas")
                        nc.any.tensor_copy(qas[:], qa[:])
                        nc.any.tensor_tensor(cwsb[:, :, fl, 0, :], qas[:, :, 0],
                                             qb[:, :, 1], ALU.subtract)
                        nc.any.tensor_tensor(cwsb[:, :, fl, 1, :], qas[:, :, 1],
                                             qb[:, :, 0], ALU.add)
                    # half-0 flush: spread one chunk-DMA per pair-iteration
                    # once freqs 0..33 are done (g>=17), collective after all 8
                    if 17 <= g <= 24:
                        flush_dma(0, g - 17)
                    if g == 25:
                        collective(0)
                for d in range(N_CORES):
                    flush_dma(1, d)
                collective(1)
            # contraction row order: r' = h*640 + s*68 + f2h (640 = 5*128 pads
            # each 544-row half to a k-block boundary); ones/bias row at 1280
            cgh = [cwx_g[h].rearrange("s f c b -> (s f) (c b)") for h in range(2)]
            cg_v = [cgh[h][0:512].rearrange("(k p) cb -> p k cb", p=128)
                    for h in range(2)]
            with tc.tile_pool(name="fn", bufs=2) as pf, \
                 tc.tile_pool(name="fnl", bufs=2) as pfl, \
                 tc.tile_pool(name="fnp", bufs=2, space="PSUM") as pfp:
                for m in range(4):
                    msl = slice(m * 128, (m + 1) * 128)
                    ltk0 = pfl.tile([128, 4, 128], BF16, tag="ltk0")
                    nc.scalar.dma_start(ltk0[:], cg_v[0][:, :, msl])
                    ltk1 = pfl.tile([128, 4, 128], BF16, tag="ltk1")
                    nc.sync.dma_start(ltk1[:], cg_v[1][:, :, msl])
                    ltr = pfl.tile([32, 2, 128], BF16, tag="ltr")
                    nc.sync.dma_start(ltr[:, 0], cgh[0][512:544, msl])
                    nc.scalar.dma_start(ltr[:, 1], cgh[1][512:544, msl])
                    ltb = pfl.tile([1, 128], BF16, tag="ltb")
                    nc.sync.dma_start(ltb[0:1, :], bias_exp[msl])
                    for n2 in range(2):
                        nsl = slice(n2 * 512, (n2 + 1) * 512)
                        psY = pfp.tile([128, 512], FP32, tag="psY")
                        for h in range(2):
                            for kc in range(4):
                                lt = (ltk0 if h == 0 else ltk1)[:, kc, :]
                                nc.tensor.matmul(psY[:], lt,
                                                 idft_sb[:, 5 * h + kc, nsl],
                                                 start=(h == 0 and kc == 0),
                                                 stop=False)
                            nc.tensor.matmul(psY[:], ltr[:, h, :],
                                             idft_sb[0:32, 5 * h + 4, nsl],
                                             start=False, stop=False)
                        nc.tensor.matmul(psY[:], ltb[:],
                                         idft_sb[0:1, 10, nsl],
                                         start=False, stop=True)
                        ysb = pf.tile([128, 512], FP32, tag="ysb")
                        nc.any.tensor_copy(ysb[:], psY[:])
                        nc.sync.dma_start(
                            y_out.rearrange("co b x -> (co b) x")[m * 128:(m + 1) * 128,
                                                                  n2 * 512:(n2 + 1) * 512],
                            ysb[:])

    nc.finalize()
    return nc


_NC_CACHE = None
_LAST_IN_MAPS = None
_LAST_RES = None


def kernel(x, weight, alpha, bias, H_real, H_imag):
    global _NC_CACHE, _LAST_IN_MAPS, _LAST_RES
    x = np.ascontiguousarray(np.asarray(x, np.float32))
    weight = np.ascontiguousarray(np.asarray(weight, np.float32))
    alpha = np.asarray(alpha, np.float32)
    bias = np.asarray(bias, np.float32)
    H_real = np.ascontiguousarray(np.asarray(H_real, np.float32))
    H_imag = np.ascontiguousarray(np.asarray(H_imag, np.float32))

    dft_c, idft, ph = _host_consts()

    # host prep: layout permutations (+ one elementwise +/-), bf16 casts
    w4 = weight.astype(np.float32)
    wm = w4 - w4.transpose(1, 0, 2, 3)            # [i,j,p,q]
    wp = w4 + w4.transpose(1, 0, 2, 3)
    wmT_np = np.ascontiguousarray(
        wm.transpose(2, 3, 1, 0).reshape(9, 65536)).astype(NPBF)
    wpT_np = np.ascontiguousarray(
        wp.transpose(2, 3, 1, 0).reshape(9, 65536)).astype(NPBF)
    wflat_np = np.ascontiguousarray(weight.reshape(128, 4608))
    xt_np = np.ascontiguousarray(
        x.reshape(B * C, PIX).T).astype(NPBF)     # [pix, b*C+c]

    # rows reordered to r' = h*640 + s*68 + (fl-34h)*2 + e  (old r =
    # s*136 + fl*2 + e); rows h*640+544..h*640+639 stay zero; ones at 1280
    idft_np = np.zeros((11 * 128, PIX), np.float32)
    old = idft.astype(np.float32)
    for h in range(2):
        for s in range(N_CORES):
            for flh in range(34):
                src = s * 136 + (34 * h + flh) * 2
                dst = h * 640 + s * 68 + flh * 2
                idft_np[dst:dst + 2] = old[src:src + 2]
    idft_np[1280] = 1.0                           # bias row
    idft_np = idft_np.astype(NPBF)

    in_maps = []
    for c in range(N_CORES):
        fsl = slice(FL * c, FL * (c + 1))
        dft_core = np.empty((PIX, F2L), np.float32)
        dft_core[:, 0::2] = dft_c[:, fsl].real
        dft_core[:, 1::2] = dft_c[:, fsl].imag
        ph_core = ph[fsl]                          # [FL, 9]
        in_maps.append({
            "xt_in": xt_np,
            "wflat": wflat_np,
            "wmT": wmT_np,
            "wpT": wpT_np,
            "alpha_in": alpha.reshape(1),
            # [f, k, j, i] -> [f, p, k, jc, i]  (j = jc*128 + p)
            "h2_in": np.ascontiguousarray(np.stack(
                [H_real[fsl].transpose(0, 2, 1),
                 H_imag[fsl].transpose(0, 2, 1)], axis=1)
                .reshape(FL, 2, 2, 128, C)
                .transpose(0, 3, 1, 2, 4)).astype(NPBF),
            "dft_in": dft_core.astype(NPBF),
            "idft_in": idft_np,
            "phre_in": np.ascontiguousarray(ph_core.real.T.astype(np.float32)),
            "nphim_in": np.ascontiguousarray((-ph_core.imag.T).astype(np.float32)),
            "bias_exp": np.repeat(bias[32 * c:32 * (c + 1)], 16).astype(NPBF),
        })

    _LAST_IN_MAPS = in_maps
    if _NC_CACHE is None:
        _NC_CACHE = _build_nc()
    res = bass_utils.run_bass_kernel_spmd(_NC_CACHE, in_maps,
                                          core_ids=list(range(N_CORES)))
    _LAST_RES = res
    y = np.empty((B, C, N, N), np.float32)
    for c in range(N_CORES):
        y[:, 32 * c:32 * (c + 1)] = res.results[c]["y_out"].reshape(
            32, B, N, N).transpose(1, 0, 2, 3)
    return y


# revision 57
# speedup vs baseline: 1.0071x; 1.0071x over previous
"""CayleyConvED Trainium2 kernel (8-core SPMD, frequency-sharded), v2.

Math (matches reference.py):
  xfft = rfft2(x)                         -> per-freq [cin, B] complex
  W[f,i,j] = sum_t w[i,j,t] * exp(+2pi*i*(u*(p-1)+v*(q-1))/32)   (t=(p,q))
  A = c*(W - W^H),  c = alpha/||W||_F     (skew-Hermitian, ||A||_2 ~ 0.04)
  Q = (I+A)^{-1}(I-A) = I - 2A + 2A^2 - ...
  v = Q@x ~= x - 2Ax  (first-order Neumann; rel err ~ 2||A||^2 ~ 1e-3)
  cwx = H v;  y = irfft2(cwx) + bias

v2 vs v1: one Neumann step instead of 3 (tolerance is 2e-2); all heavy
matmuls in bf16 (fp32 matmul issues 2 HW passes); A / H / x / idft / the
AllToAll payload staged in bf16 (halves HBM+collective traffic); the -2
factor of (I - 2A) is folded into the A planes at build time.

FFTs are dense matmuls against host-precomputed DFT basis matrices. wnorm
uses the closed form ||W||^2 = 32*(16*||w||^2 + ||sum_q w||^2)  (fp32).

Sharding: F=544 freqs = 8 cores x 68. The final inverse-FFT contraction over
all freqs uses an on-device AllToAll (each core keeps cout slice
[32c, 32c+32)), then per-core dense matmul; host concatenates cout slices.
"""

import numpy as np
import ml_dtypes

import concourse.bass as bass
import concourse.mybir as mybir
import concourse.tile as tile
from concourse import bacc
from concourse import bass_utils

N_CORES = 8
B, C, N = 16, 256, 32
NF = N // 2 + 1          # 17
F = N * NF               # 544
FL = F // N_CORES        # 68 freqs per core
F2L = 2 * FL             # 136 re/im rows per core
PIX = N * N              # 1024
FP32 = mybir.dt.float32
BF16 = mybir.dt.bfloat16
NPBF = ml_dtypes.bfloat16
ALU = mybir.AluOpType
ACTF = mybir.ActivationFunctionType


# ----------------------------------------------------------------------------
# Host-side constants (input-independent)
# ----------------------------------------------------------------------------

def _host_consts():
    # forward: rfft2 response of each pixel impulse; f = u*NF + v
    imp = np.zeros((PIX, N, N))
    imp[np.arange(PIX), np.arange(PIX) // N, np.arange(PIX) % N] = 1.0
    dft_c = np.fft.rfft2(imp).reshape(PIX, F)          # [pix, f] complex

    # inverse: irfft2 response of unit re/im at each bin -> [2F, PIX]
    basis = np.zeros((2 * F, N, NF), np.complex128)
    fidx = np.arange(F)
    basis[2 * fidx, fidx // NF, fidx % NF] = 1.0
    basis[2 * fidx + 1, fidx // NF, fidx % NF] = 1.0j
    idft = np.fft.irfft2(basis, s=(N, N)).reshape(2 * F, PIX)

    # phase basis: W[f,i,j] = sum_t w[i,j,t] ph[f,t], t = p*3+q
    u = (np.arange(F) // NF).reshape(F, 1)
    v = (np.arange(F) % NF).reshape(F, 1)
    p = (np.arange(9) // 3).reshape(1, 9)
    q = (np.arange(9) % 3).reshape(1, 9)
    ph = np.exp(2j * np.pi * (u * (p - 1) + v * (q - 1)) / N)  # [F, 9]
    return dft_c, idft, ph


# ----------------------------------------------------------------------------
# Device program (SPMD: one NEFF, per-core data via in_maps)
# ----------------------------------------------------------------------------

def _build_nc():
    nc = bacc.Bacc("TRN2", target_bir_lowering=False, debug=False,
                   enable_asserts=True, num_devices=N_CORES)

    def din(name, shape, dt=FP32):
        return nc.dram_tensor(name, shape, dt, kind="ExternalInput").ap()

    xt_in = din("xt_in", [PIX, B * C], BF16)
    wflat = din("wflat", [128, 4608])
    wmT = din("wmT", [9, 65536], BF16)
    wpT = din("wpT", [9, 65536], BF16)
    alpha_in = din("alpha_in", [1])
    h2_in = din("h2_in", [FL, 128, 2, 2, C], BF16)  # [f, p, {re,im}, jc, i]
    dft_in = din("dft_in", [PIX, F2L], BF16)
    # rows h*640+0..543 = idft half h (perm640), 1280 = ones, rest zero
    idft_in = din("idft_in", [11 * 128, PIX], BF16)
    phre_in = din("phre_in", [9, FL])
    nphim_in = din("nphim_in", [9, FL])
    bias_exp = din("bias_exp", [512], BF16)    # repeat(bias[co-slice], 16)
    y_out = nc.dram_tensor("y_out", [32, B, PIX], FP32, kind="ExternalOutput").ap()

    with tile.TileContext(nc) as tc:
        with tc.tile_pool(name="const", bufs=1) as pc, \
             tc.tile_pool(name="dram", bufs=1, space="DRAM") as pdram:
            dft_sb = pc.tile([128, 8, F2L], BF16)
            nc.sync.dma_start(dft_sb[:], dft_in.rearrange("(k p) f -> p k f", p=128))
            idft_sb = pc.tile([128, 11, PIX], BF16)
            nc.sync.dma_start(idft_sb[:], idft_in.rearrange("(k p) f -> p k f", p=128))
            ones9 = pc.tile([128, 9], FP32)
            nc.vector.memset(ones9[:], 1.0)
            phre_b = pc.tile([9, FL], BF16)    # scaled by -2c
            nphim_b = pc.tile([9, FL], BF16)
            # xfft staging: [cin%128, cchunk, f2(local), b]  fp32 + bf16
            xfft_f = pc.tile([128, 2, F2L, 16], FP32)
            xfft_b = pc.tile([128, 2, F2L, 16], BF16)

            # [f, p, jc, plane, i]: reader tile [128p, jc, plane, i] is fully
            # contiguous per partition (2KB runs)
            abuf = pdram.tile([FL, 128, 2, 2, 256], BF16)   # -2*A_re / +2*A_im
            # two half-tensors so each AllToAll is a full-tensor op; half 0's
            # exchange overlaps the second half of the main loop
            cwx_l = [pdram.tile([N_CORES, FL, 32, 16], BF16, name=f"cwx_l{h}")
                     for h in range(2)]
            cwx_g = [pdram.tile([N_CORES, FL, 32, 16], BF16, name=f"cwx_g{h}")
                     for h in range(2)]

            # ---------------- wnorm -> c, scale phase tables ----------------
            with tc.tile_pool(name="wn", bufs=1) as pw, \
                 tc.tile_pool(name="wnp", bufs=1, space="PSUM") as pwp:
                w_sb = pw.tile([128, 512, 3, 3], FP32)
                nc.sync.dma_start(w_sb[:], wflat.rearrange("p (c u v) -> p c u v", u=3, v=3))
                w_fl = w_sb.rearrange("p a u v -> p (a u v)")
                sq = pw.tile([128, 4608], FP32)
                nc.vector.tensor_tensor(sq[:], w_fl, w_fl, ALU.mult)
                acc1 = pw.tile([128, 1], FP32)
                nc.vector.tensor_reduce(acc1[:], sq[:], mybir.AxisListType.X, ALU.add)
                s3 = pw.tile([128, 512, 3], FP32)
                nc.vector.tensor_tensor(s3[:], w_sb[:, :, :, 0], w_sb[:, :, :, 1], ALU.add)
                nc.vector.tensor_tensor(s3[:], s3[:], w_sb[:, :, :, 2], ALU.add)
                sq2 = pw.tile([128, 512, 3], FP32)
                nc.vector.tensor_tensor(sq2[:], s3[:], s3[:], ALU.mult)
                acc2 = pw.tile([128, 1], FP32)
                nc.vector.tensor_reduce(acc2[:], sq2.rearrange("p a b -> p (a b)"),
                                        mybir.AxisListType.X, ALU.add)
                tot = pw.tile([128, 1], FP32)
                nc.vector.tensor_scalar(tot[:], acc1[:], 16.0, None, ALU.mult)
                nc.vector.tensor_tensor(tot[:], tot[:], acc2[:], ALU.add)
                s9p = pwp.tile([9, 1], FP32)
                nc.tensor.matmul(s9p[:], ones9[:, 0:9], tot[:], start=True, stop=True)
                s9 = pw.tile([9, 1], FP32)
                nc.vector.tensor_copy(s9[:], s9p[:])
                rt9 = pw.tile([9, 1], FP32)
                nc.scalar.activation(rt9[:], s9[:], ACTF.Sqrt, bias=0.0, scale=32.0)
                rc9 = pw.tile([9, 1], FP32)
                nc.vector.reciprocal(rc9[:], rt9[:])
                al9 = pw.tile([9, 1], FP32)
                for t in range(9):
                    nc.sync.dma_start(al9[t:t + 1, 0:1], alpha_in[0:1])
                c9 = pw.tile([9, 1], FP32)
                nc.vector.tensor_tensor(c9[:], rc9[:], al9[:], ALU.mult)
                nc.vector.tensor_scalar(c9[:], c9[:], -2.0, None, ALU.mult)
                phre_r = pw.tile([9, FL], FP32)
                nphim_r = pw.tile([9, FL], FP32)
                nc.sync.dma_start(phre_r[:], phre_in[:])
                nc.sync.dma_start(nphim_r[:], nphim_in[:])
                phre_s = pw.tile([9, FL], FP32)
                nphim_s = pw.tile([9, FL], FP32)
                nc.vector.tensor_scalar(phre_s[:], phre_r[:], c9[:, 0:1], None, ALU.mult)
                nc.vector.tensor_scalar(nphim_s[:], nphim_r[:], c9[:, 0:1], None, ALU.mult)
                nc.vector.tensor_copy(phre_b[:], phre_s[:])
                nc.vector.tensor_copy(nphim_b[:], nphim_s[:])

            # --- A build: abuf[plane, f, j*256+i] = {-2A_re, +2A_im}[i,j] ---
            with tc.tile_pool(name="ab", bufs=2) as pa, \
                 tc.tile_pool(name="abp", bufs=4, space="PSUM") as pap:
                for e8 in range(8):
                    wm8 = pa.tile([9, 8192], BF16, tag="wm8")
                    wp8 = pa.tile([9, 8192], BF16, tag="wp8")
                    nc.sync.dma_start(wm8[:], wmT[:, e8 * 8192:(e8 + 1) * 8192])
                    nc.sync.dma_start(wp8[:], wpT[:, e8 * 8192:(e8 + 1) * 8192])
                    sbA8 = pa.tile([FL, 2, 8192], BF16, tag="sbA8")
                    for c2 in range(16):
                        for plane, (ph_t, rhs_t) in enumerate(
                                ((phre_b, wm8), (nphim_b, wp8))):
                            psA = pap.tile([FL, 512], FP32, tag="psA")
                            nc.tensor.matmul(psA[:], ph_t[:],
                                             rhs_t[:, c2 * 512:(c2 + 1) * 512],
                                             start=True, stop=True)
                            nc.vector.tensor_copy(
                                sbA8[:, plane, c2 * 512:(c2 + 1) * 512], psA[:])
                    for plane in range(2):
                        nc.scalar.dma_start(
                            abuf[:, (e8 % 4) * 32:(e8 % 4 + 1) * 32,
                                 e8 // 4, plane, :],
                            sbA8[:, plane, :].rearrange("f (j i) -> f j i", i=256))

            # ---------------- forward FFT of x ----------------
            xt_v = xt_in.rearrange("(k p) c -> p k c", p=128)
            with tc.tile_pool(name="xf", bufs=4) as px, \
                 tc.tile_pool(name="xfo", bufs=2, space="PSUM") as pxo:
                for b in range(B):
                    xall = px.tile([128, 8, 256], BF16, tag="xall")
                    nc.sync.dma_start(xall[:], xt_v[:, :, b * C:(b + 1) * C])
                    for cc in range(2):
                        psX = pxo.tile([128, F2L], FP32, tag="psX")
                        for k in range(8):
                            nc.tensor.matmul(psX[:],
                                             xall[:, k, cc * 128:cc * 128 + 128],
                                             dft_sb[:, k, :],
                                             start=(k == 0), stop=(k == 7))
                        nc.vector.tensor_copy(xfft_f[:, cc, :, b], psX[:])
                        nc.vector.tensor_copy(xfft_b[:, cc, :, b], psX[:])

            # ---------------- per-frequency main loop ----------------
            # pa = (-2A_re) @ [xre,xim], pb = (+2A_im) @ [xre,xim]
            # v_re = xre + pa0 + pb1 ; v_im = xim + pa1 - pb0   (v = x - 2Ax)
            # qa = Hre @ v, qb = Him @ v ; cw = (qa0-qb1, qa1+qb0)
            # cw accumulates in SBUF [p, cc, f, e, b]; flushed to DRAM in
            # per-dest-core chunks per f-half, each half's AllToAll overlapping
            # the second half of the main loop.
            cwsb = pc.tile([128, 2, FL, 2, 16], BF16)

            def flush_dma(hs, d):
                src = cwsb[(d % 4) * 32:(d % 4 + 1) * 32, d // 4,
                           hs * 34:(hs + 1) * 34].rearrange("p f e b -> p (f e) b")
                dst = cwx_l[hs][d].rearrange("f2 c b -> c f2 b")
                (nc.scalar if d % 2 else nc.sync).dma_start(dst, src)

            def collective(hs):
                nc.gpsimd.collective_compute(
                    "AllToAll", ALU.bypass,
                    replica_groups=[list(range(N_CORES))],
                    ins=[cwx_l[hs].opt()], outs=[cwx_g[hs].opt()],
                )

            with tc.tile_pool(name="mn", bufs=4) as pm, \
                 tc.tile_pool(name="mz", bufs=4) as pz, \
                 tc.tile_pool(name="mp", bufs=4, space="PSUM") as pmp, \
                 tc.tile_pool(name="mq", bufs=4, space="PSUM") as pmq:
                for g in range(FL // 2):
                    a8 = pm.tile([128, 2, 2, 2, 256], BF16, tag="a8")
                    nc.scalar.dma_start(
                        a8[:], abuf.rearrange("f p jc pl i -> p f jc pl i")
                        [:, 2 * g:2 * g + 2])
                    h8 = pm.tile([128, 2, 2, 2, 256], BF16, tag="h8")
                    nc.sync.dma_start(
                        h8[:], h2_in.rearrange("f p k c i -> p f k c i")
                        [:, 2 * g:2 * g + 2])
                    for q in range(2):
                        fl = 2 * g + q
                        a4 = a8[:, q]                              # [p,jc,pl,i]
                        h4 = h8[:, q]                              # [p,k,jc,i]
                        x2b = xfft_b[:, :, 2 * fl:2 * fl + 2, :]   # bf16
                        x2f = xfft_f[:, :, 2 * fl:2 * fl + 2, :]   # fp32
                        pab = pmp.tile([128, 2, 2, 2, 16], FP32, tag="pab")
                        pa_, pb_ = pab[:, 0], pab[:, 1]
                        for ic in range(2):
                            for jc in range(2):
                                nc.tensor.matmul(pa_[:, ic],
                                                 a4[:, jc, 0, ic * 128:(ic + 1) * 128],
                                                 x2b[:, jc], start=(jc == 0),
                                                 stop=(jc == 1))
                        for ic in range(2):
                            for jc in range(2):
                                nc.tensor.matmul(pb_[:, ic],
                                                 a4[:, jc, 1, ic * 128:(ic + 1) * 128],
                                                 x2b[:, jc], start=(jc == 0),
                                                 stop=(jc == 1))
                        tmp = pz.tile([128, 2, 2, 16], FP32, tag="vt")
                        v = pz.tile([128, 2, 2, 16], BF16, tag="v")
                        nc.any.tensor_tensor(tmp[:], x2f, pa_[:], ALU.add)
                        nc.any.tensor_tensor(v[:, :, 0], tmp[:, :, 0],
                                             pb_[:, :, 1], ALU.add)
                        nc.any.tensor_tensor(v[:, :, 1], tmp[:, :, 1],
                                             pb_[:, :, 0], ALU.subtract)
                        qab = pmq.tile([128, 2, 2, 2, 16], FP32, tag="qab")
                        qa, qb = qab[:, 0], qab[:, 1]
                        for ic in range(2):
                            for jc in range(2):
                                nc.tensor.matmul(qa[:, ic],
                                                 h4[:, 0, jc, ic * 128:(ic + 1) * 128],
                                                 v[:, jc], start=(jc == 0),
                                                 stop=(jc == 1))
                        for ic in range(2):
                            for jc in range(2):
                                nc.tensor.matmul(qb[:, ic],
                                                 h4[:, 1, jc, ic * 128:(ic + 1) * 128],
                                                 v[:, jc], start=(jc == 0),
                                                 stop=(jc == 1))
                        qas = pz.tile([128, 2, 2, 16], FP32, tag="qas")
                        nc.any.tensor_copy(qas[:], qa[:])
                        nc.any.tensor_tensor(cwsb[:, :, fl, 0, :], qas[:, :, 0],
                                             qb[:, :, 1], ALU.subtract)
                        nc.any.tensor_tensor(cwsb[:, :, fl, 1, :], qas[:, :, 1],
                                             qb[:, :, 0], ALU.add)
                    # half-0 flush: spread one chunk-DMA per pair-iteration
                    # once freqs 0..33 are done (g>=17), collective after all 8
                    if 17 <= g <= 24:
                        flush_dma(0, g - 17)
                    if g == 25:
                        collective(0)
                for d in range(N_CORES):
                    flush_dma(1, d)
                collective(1)
            # contraction row order: r' = h*640 + s*68 + f2h (640 = 5*128 pads
            # each 544-row half to a k-block boundary); ones/bias row at 1280
            cgh = [cwx_g[h].rearrange("s f c b -> (s f) (c b)") for h in range(2)]
            cg_v = [cgh[h][0:512].rearrange("(k p) cb -> p k cb", p=128)
                    for h in range(2)]
            with tc.tile_pool(name="fn", bufs=2) as pf, \
                 tc.tile_pool(name="fnl", bufs=2) as pfl, \
                 tc.tile_pool(name="fnp", bufs=2, space="PSUM") as pfp:
                for m in range(4):
                    msl = slice(m * 128, (m + 1) * 128)
                    ltk0 = pfl.tile([128, 4, 128], BF16, tag="ltk0")
                    nc.scalar.dma_start(ltk0[:], cg_v[0][:, :, msl])
                    ltk1 = pfl.tile([128, 4, 128], BF16, tag="ltk1")
                    nc.sync.dma_start(ltk1[:], cg_v[1][:, :, msl])
                    ltr = pfl.tile([32, 2, 128], BF16, tag="ltr")
                    nc.sync.dma_start(ltr[:, 0], cgh[0][512:544, msl])
                    nc.scalar.dma_start(ltr[:, 1], cgh[1][512:544, msl])
                    ltb = pfl.tile([1, 128], BF16, tag="ltb")
                    nc.sync.dma_start(ltb[0:1, :], bias_exp[msl])
                    for n2 in range(2):
                        nsl = slice(n2 * 512, (n2 + 1) * 512)
                        psY = pfp.tile([128, 512], FP32, tag="psY")
                        for h in range(2):
                            for kc in range(4):
                                lt = (ltk0 if h == 0 else ltk1)[:, kc, :]
                                nc.tensor.matmul(psY[:], lt,
                                                 idft_sb[:, 5 * h + kc, nsl],
                                                 start=(h == 0 and kc == 0),
                                                 stop=False)
                            nc.tensor.matmul(psY[:], ltr[:, h, :],
                                             idft_sb[0:32, 5 * h + 4, nsl],
                                             start=False, stop=False)
                        nc.tensor.matmul(psY[:], ltb[:],
                                         idft_sb[0:1, 10, nsl],
                                         start=False, stop=True)
                        ysb = pf.tile([128, 512], FP32, tag="ysb")
                        nc.any.tensor_copy(ysb[:], psY[:])
                        nc.sync.dma_start(
                            y_out.rearrange("co b x -> (co b) x")[m * 128:(m + 1) * 128,
                                                                  n2 * 512:(n2 + 1) * 512],
                            ysb[:])

    nc.finalize()
    return nc


_NC_CACHE = None
_LAST_IN_MAPS = None
_LAST_RES = None


def kernel(x, weight, alpha, bias, H_real, H_imag):
    global _NC_CACHE, _LAST_IN_MAPS, _LAST_RES
    x = np.ascontiguousarray(np.asarray(x, np.float32))
    weight = np.ascontiguousarray(np.asarray(weight, np.float32))
    alpha = np.asarray(alpha, np.float32)
    bias = np.asarray(bias, np.float32)
    H_real = np.ascontiguousarray(np.asarray(H_real, np.float32))
    H_imag = np.ascontiguousarray(np.asarray(H_imag, np.float32))

    dft_c, idft, ph = _host_consts()

    # host prep: layout permutations (+ one elementwise +/-), bf16 casts
    w4 = weight.astype(np.float32)
    wm = w4 - w4.transpose(1, 0, 2, 3)            # [i,j,p,q]
    wp = w4 + w4.transpose(1, 0, 2, 3)
    wmT_np = np.ascontiguousarray(
        wm.transpose(2, 3, 1, 0).reshape(9, 65536)).astype(NPBF)
    wpT_np = np.ascontiguousarray(
        wp.transpose(2, 3, 1, 0).reshape(9, 65536)).astype(NPBF)
    wflat_np = np.ascontiguousarray(weight.reshape(128, 4608))
    xt_np = np.ascontiguousarray(
        x.reshape(B * C, PIX).T).astype(NPBF)     # [pix, b*C+c]

    # rows reordered to r' = h*640 + s*68 + (fl-34h)*2 + e  (old r =
    # s*136 + fl*2 + e); rows h*640+544..h*640+639 stay zero; ones at 1280
    idft_np = np.zeros((11 * 128, PIX), np.float32)
    old = idft.astype(np.float32)
    for h in range(2):
        for s in range(N_CORES):
            for flh in range(34):
                src = s * 136 + (34 * h + flh) * 2
                dst = h * 640 + s * 68 + flh * 2
                idft_np[dst:dst + 2] = old[src:src + 2]
    idft_np[1280] = 1.0                           # bias row
    idft_np = idft_np.astype(NPBF)

    in_maps = []
    for c in range(N_CORES):
        fsl = slice(FL * c, FL * (c + 1))
        dft_core = np.empty((PIX, F2L), np.float32)
        dft_core[:, 0::2] = dft_c[:, fsl].real
        dft_core[:, 1::2] = dft_c[:, fsl].imag
        ph_core = ph[fsl]                          # [FL, 9]
        in_maps.append({
            "xt_in": xt_np,
            "wflat": wflat_np,
            "wmT": wmT_np,
            "wpT": wpT_np,
            "alpha_in": alpha.reshape(1),
            # [f, k, j, i] -> [f, p, k, jc, i]  (j = jc*128 + p)
            "h2_in": np.ascontiguousarray(np.stack(
                [H_real[fsl].transpose(0, 2, 1),
                 H_imag[fsl].transpose(0, 2, 1)], axis=1)
                .reshape(FL, 2, 2, 128, C)
                .transpose(0, 3, 1, 2, 4)).astype(NPBF),
            "dft_in": dft_core.astype(NPBF),
            "idft_in": idft_np,
            "phre_in": np.ascontiguousarray(ph_core.real.T.astype(np.float32)),
            "nphim_in": np.ascontiguousarray((-ph_core.imag.T).astype(np.float32)),
            "bias_exp": np.repeat(bias[32 * c:32 * (c + 1)], 16).astype(NPBF),
        })

    _LAST_IN_MAPS = in_maps
    if _NC_CACHE is None:
        _NC_CACHE = _build_nc()
    res = bass_utils.run_bass_kernel_spmd(_NC_CACHE, in_maps,
                                          core_ids=list(range(N_CORES)))
    _LAST_RES = res
    y = np.empty((B, C, N, N), np.float32)
    for c in range(N_CORES):
        y[:, 32 * c:32 * (c + 1)] = res.results[c]["y_out"].reshape(
            32, B, N, N).transpose(1, 0, 2, 3)
    return y


# revision 63
# speedup vs baseline: 1.0205x; 1.0134x over previous
"""CayleyConvED Trainium2 kernel (8-core SPMD, frequency-sharded), v2.

Math (matches reference.py):
  xfft = rfft2(x)                         -> per-freq [cin, B] complex
  W[f,i,j] = sum_t w[i,j,t] * exp(+2pi*i*(u*(p-1)+v*(q-1))/32)   (t=(p,q))
  A = c*(W - W^H),  c = alpha/||W||_F     (skew-Hermitian, ||A||_2 ~ 0.04)
  Q = (I+A)^{-1}(I-A) = I - 2A + 2A^2 - ...
  v = Q@x ~= x - 2Ax  (first-order Neumann; rel err ~ 2||A||^2 ~ 1e-3)
  cwx = H v;  y = irfft2(cwx) + bias

v2 vs v1: one Neumann step instead of 3 (tolerance is 2e-2); all heavy
matmuls in bf16 (fp32 matmul issues 2 HW passes); A / H / x / idft / the
AllToAll payload staged in bf16 (halves HBM+collective traffic); the -2
factor of (I - 2A) is folded into the A planes at build time.

FFTs are dense matmuls against host-precomputed DFT basis matrices. wnorm
uses the closed form ||W||^2 = 32*(16*||w||^2 + ||sum_q w||^2)  (fp32).

Sharding: F=544 freqs = 8 cores x 68. The final inverse-FFT contraction over
all freqs uses an on-device AllToAll (each core keeps cout slice
[32c, 32c+32)), then per-core dense matmul; host concatenates cout slices.
"""

import numpy as np
import ml_dtypes

import concourse.bass as bass
import concourse.mybir as mybir
import concourse.tile as tile
from concourse import bacc
from concourse import bass_utils

N_CORES = 8
B, C, N = 16, 256, 32
NF = N // 2 + 1          # 17
F = N * NF               # 544
FL = F // N_CORES        # 68 freqs per core
F2L = 2 * FL             # 136 re/im rows per core
PIX = N * N              # 1024
FP32 = mybir.dt.float32
BF16 = mybir.dt.bfloat16
NPBF = ml_dtypes.bfloat16
ALU = mybir.AluOpType
ACTF = mybir.ActivationFunctionType


# ----------------------------------------------------------------------------
# Host-side constants (input-independent)
# ----------------------------------------------------------------------------

def _host_consts():
    # forward: rfft2 response of each pixel impulse; f = u*NF + v
    imp = np.zeros((PIX, N, N))
    imp[np.arange(PIX), np.arange(PIX) // N, np.arange(PIX) % N] = 1.0
    dft_c = np.fft.rfft2(imp).reshape(PIX, F)          # [pix, f] complex

    # inverse: irfft2 response of unit re/im at each bin -> [2F, PIX]
    basis = np.zeros((2 * F, N, NF), np.complex128)
    fidx = np.arange(F)
    basis[2 * fidx, fidx // NF, fidx % NF] = 1.0
    basis[2 * fidx + 1, fidx // NF, fidx % NF] = 1.0j
    idft = np.fft.irfft2(basis, s=(N, N)).reshape(2 * F, PIX)

    # phase basis: W[f,i,j] = sum_t w[i,j,t] ph[f,t], t = p*3+q
    u = (np.arange(F) // NF).reshape(F, 1)
    v = (np.arange(F) % NF).reshape(F, 1)
    p = (np.arange(9) // 3).reshape(1, 9)
    q = (np.arange(9) % 3).reshape(1, 9)
    ph = np.exp(2j * np.pi * (u * (p - 1) + v * (q - 1)) / N)  # [F, 9]
    return dft_c, idft, ph


# ----------------------------------------------------------------------------
# Device program (SPMD: one NEFF, per-core data via in_maps)
# ----------------------------------------------------------------------------

def _build_nc():
    nc = bacc.Bacc("TRN2", target_bir_lowering=False, debug=False,
                   enable_asserts=True, num_devices=N_CORES)

    def din(name, shape, dt=FP32):
        return nc.dram_tensor(name, shape, dt, kind="ExternalInput").ap()

    xt_in = din("xt_in", [PIX, B * C], BF16)
    wflat = din("wflat", [128, 4608])
    wmT = din("wmT", [9, 65536], BF16)
    wpT = din("wpT", [9, 65536], BF16)
    alpha_in = din("alpha_in", [1])
    h2_in = din("h2_in", [FL, 128, 2, 2, C], BF16)  # [f, p, {re,im}, jc, i]
    dft_in = din("dft_in", [PIX, F2L], BF16)
    idft_in = din("idft_in", [9 * 128, PIX], BF16)  # rows 0..1087=idft, 1088=ones
    phre_in = din("phre_in", [9, FL])
    nphim_in = din("nphim_in", [9, FL])
    bias_exp = din("bias_exp", [512], BF16)    # repeat(bias[co-slice], 16)
    y_out = nc.dram_tensor("y_out", [32, B, PIX], FP32, kind="ExternalOutput").ap()

    with tile.TileContext(nc) as tc:
        with tc.tile_pool(name="const", bufs=1) as pc, \
             tc.tile_pool(name="dram", bufs=1, space="DRAM") as pdram:
            dft_sb = pc.tile([128, 8, F2L], BF16)
            nc.sync.dma_start(dft_sb[:], dft_in.rearrange("(k p) f -> p k f", p=128))
            idft_sb = pc.tile([128, 9, PIX], BF16)
            nc.sync.dma_start(idft_sb[:], idft_in.rearrange("(k p) f -> p k f", p=128))
            ones9 = pc.tile([128, 9], FP32)
            nc.vector.memset(ones9[:], 1.0)
            phre_b = pc.tile([9, FL], BF16)    # scaled by -2c
            nphim_b = pc.tile([9, FL], BF16)
            # xfft staging: [cin%128, cchunk, f2(local), b]  fp32 + bf16
            xfft_f = pc.tile([128, 2, F2L, 16], FP32)
            xfft_b = pc.tile([128, 2, F2L, 16], BF16)

            # [f, p, jc, plane, i]: reader tile [128p, jc, plane, i] is fully
            # contiguous per partition (2KB runs)
            abuf = pdram.tile([FL, 128, 2, 2, 256], BF16)   # -2*A_re / +2*A_im
            cwx_l = pdram.tile([N_CORES, F2L, 32, 16], BF16)   # dest-core major
            cwx_g = pdram.tile([N_CORES, F2L, 32, 16], BF16)   # post-AllToAll

            # ---------------- wnorm -> c, scale phase tables ----------------
            with tc.tile_pool(name="wn", bufs=1) as pw, \
                 tc.tile_pool(name="wnp", bufs=1, space="PSUM") as pwp:
                w_sb = pw.tile([128, 512, 3, 3], FP32)
                nc.sync.dma_start(w_sb[:], wflat.rearrange("p (c u v) -> p c u v", u=3, v=3))
                w_fl = w_sb.rearrange("p a u v -> p (a u v)")
                sq = pw.tile([128, 4608], FP32)
                nc.vector.tensor_tensor(sq[:], w_fl, w_fl, ALU.mult)
                acc1 = pw.tile([128, 1], FP32)
                nc.vector.tensor_reduce(acc1[:], sq[:], mybir.AxisListType.X, ALU.add)
                s3 = pw.tile([128, 512, 3], FP32)
                nc.vector.tensor_tensor(s3[:], w_sb[:, :, :, 0], w_sb[:, :, :, 1], ALU.add)
                nc.vector.tensor_tensor(s3[:], s3[:], w_sb[:, :, :, 2], ALU.add)
                sq2 = pw.tile([128, 512, 3], FP32)
                nc.vector.tensor_tensor(sq2[:], s3[:], s3[:], ALU.mult)
                acc2 = pw.tile([128, 1], FP32)
                nc.vector.tensor_reduce(acc2[:], sq2.rearrange("p a b -> p (a b)"),
                                        mybir.AxisListType.X, ALU.add)
                tot = pw.tile([128, 1], FP32)
                nc.vector.tensor_scalar(tot[:], acc1[:], 16.0, None, ALU.mult)
                nc.vector.tensor_tensor(tot[:], tot[:], acc2[:], ALU.add)
                s9p = pwp.tile([9, 1], FP32)
                nc.tensor.matmul(s9p[:], ones9[:, 0:9], tot[:], start=True, stop=True)
                s9 = pw.tile([9, 1], FP32)
                nc.vector.tensor_copy(s9[:], s9p[:])
                rt9 = pw.tile([9, 1], FP32)
                nc.scalar.activation(rt9[:], s9[:], ACTF.Sqrt, bias=0.0, scale=32.0)
                rc9 = pw.tile([9, 1], FP32)
                nc.vector.reciprocal(rc9[:], rt9[:])
                al9 = pw.tile([9, 1], FP32)
                for t in range(9):
                    nc.sync.dma_start(al9[t:t + 1, 0:1], alpha_in[0:1])
                c9 = pw.tile([9, 1], FP32)
                nc.vector.tensor_tensor(c9[:], rc9[:], al9[:], ALU.mult)
                nc.vector.tensor_scalar(c9[:], c9[:], -2.0, None, ALU.mult)
                phre_r = pw.tile([9, FL], FP32)
                nphim_r = pw.tile([9, FL], FP32)
                nc.sync.dma_start(phre_r[:], phre_in[:])
                nc.sync.dma_start(nphim_r[:], nphim_in[:])
                phre_s = pw.tile([9, FL], FP32)
                nphim_s = pw.tile([9, FL], FP32)
                nc.vector.tensor_scalar(phre_s[:], phre_r[:], c9[:, 0:1], None, ALU.mult)
                nc.vector.tensor_scalar(nphim_s[:], nphim_r[:], c9[:, 0:1], None, ALU.mult)
                nc.vector.tensor_copy(phre_b[:], phre_s[:])
                nc.vector.tensor_copy(nphim_b[:], nphim_s[:])

            # --- A build: abuf[plane, f, j*256+i] = {-2A_re, +2A_im}[i,j] ---
            with tc.tile_pool(name="ab", bufs=2) as pa, \
                 tc.tile_pool(name="abp", bufs=4, space="PSUM") as pap:
                for e8 in range(8):
                    wm8 = pa.tile([9, 8192], BF16, tag="wm8")
                    wp8 = pa.tile([9, 8192], BF16, tag="wp8")
                    nc.sync.dma_start(wm8[:], wmT[:, e8 * 8192:(e8 + 1) * 8192])
                    nc.scalar.dma_start(wp8[:], wpT[:, e8 * 8192:(e8 + 1) * 8192])
                    sbA8 = pa.tile([FL, 2, 8192], BF16, tag="sbA8")
                    for c2 in range(16):
                        for plane, (ph_t, rhs_t) in enumerate(
                                ((phre_b, wm8), (nphim_b, wp8))):
                            psA = pap.tile([FL, 512], FP32, tag="psA")
                            nc.tensor.matmul(psA[:], ph_t[:],
                                             rhs_t[:, c2 * 512:(c2 + 1) * 512],
                                             start=True, stop=True)
                            nc.vector.tensor_copy(
                                sbA8[:, plane, c2 * 512:(c2 + 1) * 512], psA[:])
                    for plane in range(2):
                        (nc.sync if (e8 + plane) % 2 else nc.scalar).dma_start(
                            abuf[:, (e8 % 4) * 32:(e8 % 4 + 1) * 32,
                                 e8 // 4, plane, :],
                            sbA8[:, plane, :].rearrange("f (j i) -> f j i", i=256))

            # ---------------- forward FFT of x ----------------
            xt_v = xt_in.rearrange("(k p) c -> p k c", p=128)
            with tc.tile_pool(name="xf", bufs=4) as px, \
                 tc.tile_pool(name="xfo", bufs=2, space="PSUM") as pxo:
                for b in range(B):
                    xall = px.tile([128, 8, 256], BF16, tag="xall")
                    (nc.scalar if b % 2 else nc.sync).dma_start(
                        xall[:], xt_v[:, :, b * C:(b + 1) * C])
                    for cc in range(2):
                        psX = pxo.tile([128, F2L], FP32, tag="psX")
                        for k in range(8):
                            nc.tensor.matmul(psX[:],
                                             xall[:, k, cc * 128:cc * 128 + 128],
                                             dft_sb[:, k, :],
                                             start=(k == 0), stop=(k == 7))
                        nc.vector.tensor_copy(xfft_f[:, cc, :, b], psX[:])
                        nc.scalar.activation(xfft_b[:, cc, :, b], psX[:], ACTF.Copy)

            # ---------------- per-frequency main loop ----------------
            # pa = (-2A_re) @ [xre,xim], pb = (+2A_im) @ [xre,xim]
            # v_re = xre + pa0 + pb1 ; v_im = xim + pa1 - pb0   (v = x - 2Ax)
            # qa = Hre @ v, qb = Him @ v ; cw = (qa0-qb1, qa1+qb0)
            # cw accumulates in SBUF [p, cc, f, e, b]; flushed to DRAM in
            # per-dest-core chunks per f-half, each half's AllToAll overlapping
            # the second half of the main loop.
            cwsb = pc.tile([128, 2, FL, 2, 16], BF16)

            def flush_half(hs):
                for d in range(N_CORES):
                    src = cwsb[(d % 4) * 32:(d % 4 + 1) * 32, d // 4,
                               hs * 34:(hs + 1) * 34].rearrange(
                                   "p f e b -> p (f e) b")
                    dst = cwx_l[d, hs * 68:(hs + 1) * 68].rearrange(
                        "f2 c b -> c f2 b")
                    (nc.scalar if d % 2 else nc.sync).dma_start(dst, src)

            with tc.tile_pool(name="mn", bufs=4) as pm, \
                 tc.tile_pool(name="mz", bufs=4) as pz, \
                 tc.tile_pool(name="mp", bufs=2, space="PSUM") as pmp, \
                 tc.tile_pool(name="mq", bufs=2, space="PSUM") as pmq:
                for g in range(FL // 2):
                    a8 = pm.tile([128, 2, 2, 2, 256], BF16, tag="a8")
                    nc.scalar.dma_start(
                        a8[:], abuf.rearrange("f p jc pl i -> p f jc pl i")
                        [:, 2 * g:2 * g + 2])
                    h8 = pm.tile([128, 2, 2, 2, 256], BF16, tag="h8")
                    nc.sync.dma_start(
                        h8[:], h2_in.rearrange("f p k c i -> p f k c i")
                        [:, 2 * g:2 * g + 2])
                    for q in range(2):
                        fl = 2 * g + q
                        a4 = a8[:, q]                              # [p,jc,pl,i]
                        h4 = h8[:, q]                              # [p,k,jc,i]
                        x2b = xfft_b[:, :, 2 * fl:2 * fl + 2, :]   # bf16
                        x2f = xfft_f[:, :, 2 * fl:2 * fl + 2, :]   # fp32
                        pa_ = pmp.tile([128, 2, 2, 16], FP32, tag="pa")
                        pb_ = pmp.tile([128, 2, 2, 16], FP32, tag="pb")
                        for ic in range(2):
                            for jc in range(2):
                                nc.tensor.matmul(pa_[:, ic],
                                                 a4[:, jc, 0, ic * 128:(ic + 1) * 128],
                                                 x2b[:, jc], start=(jc == 0),
                                                 stop=(jc == 1))
                        for ic in range(2):
                            for jc in range(2):
                                nc.tensor.matmul(pb_[:, ic],
                                                 a4[:, jc, 1, ic * 128:(ic + 1) * 128],
                                                 x2b[:, jc], start=(jc == 0),
                                                 stop=(jc == 1))
                        tmp = pz.tile([128, 2, 2, 16], FP32, tag="vt")
                        v = pz.tile([128, 2, 2, 16], BF16, tag="v")
                        nc.any.tensor_tensor(tmp[:], x2f, pa_[:], ALU.add)
                        nc.any.tensor_tensor(v[:, :, 0], tmp[:, :, 0],
                                             pb_[:, :, 1], ALU.add)
                        nc.any.tensor_tensor(v[:, :, 1], tmp[:, :, 1],
                                             pb_[:, :, 0], ALU.subtract)
                        qa = pmq.tile([128, 2, 2, 16], FP32, tag="qa")
                        qb = pmq.tile([128, 2, 2, 16], FP32, tag="qb")
                        for ic in range(2):
                            for jc in range(2):
                                nc.tensor.matmul(qa[:, ic],
                                                 h4[:, 0, jc, ic * 128:(ic + 1) * 128],
                                                 v[:, jc], start=(jc == 0),
                                                 stop=(jc == 1))
                        for ic in range(2):
                            for jc in range(2):
                                nc.tensor.matmul(qb[:, ic],
                                                 h4[:, 1, jc, ic * 128:(ic + 1) * 128],
                                                 v[:, jc], start=(jc == 0),
                                                 stop=(jc == 1))
                        qas = pz.tile([128, 2, 2, 16], FP32, tag="qas")
                        nc.any.tensor_copy(qas[:], qa[:])
                        nc.any.tensor_tensor(cwsb[:, :, fl, 0, :], qas[:, :, 0],
                                             qb[:, :, 1], ALU.subtract)
                        nc.any.tensor_tensor(cwsb[:, :, fl, 1, :], qas[:, :, 1],
                                             qb[:, :, 0], ALU.add)
                    if g == FL // 4 - 1:
                        flush_half(0)
                flush_half(1)
            nc.gpsimd.collective_compute(
                "AllToAll", ALU.bypass,
                replica_groups=[list(range(N_CORES))],
                ins=[cwx_l.opt()], outs=[cwx_g.opt()],
            )
            cg = cwx_g.rearrange("s f c b -> (s f) (c b)")   # [1088, 512]
            cg_v = cg[0:1024].rearrange("(k p) cb -> p k cb", p=128)
            with tc.tile_pool(name="fn", bufs=2) as pf, \
                 tc.tile_pool(name="fnl", bufs=2) as pfl, \
                 tc.tile_pool(name="fnp", bufs=2, space="PSUM") as pfp:
                for m in range(4):
                    ltk = pfl.tile([128, 8, 128], BF16, tag="ltk")
                    nc.scalar.dma_start(ltk[:], cg_v[:, :, m * 128:(m + 1) * 128])
                    lt8 = pfl.tile([65, 128], BF16, tag="lt8")
                    nc.sync.dma_start(
                        lt8[0:64, :], cg[1024:1088, m * 128:(m + 1) * 128])
                    nc.sync.dma_start(
                        lt8[64:65, :], bias_exp[m * 128:(m + 1) * 128])
                    for n2 in range(2):
                        psY = pfp.tile([128, 512], FP32, tag="psY")
                        for kc in range(8):
                            nc.tensor.matmul(psY[:], ltk[:, kc, :],
                                             idft_sb[:, kc, n2 * 512:(n2 + 1) * 512],
                                             start=(kc == 0), stop=False)
                        nc.tensor.matmul(psY[:], lt8[:],
                                         idft_sb[0:65, 8, n2 * 512:(n2 + 1) * 512],
                                         start=False, stop=True)
                        ysb = pf.tile([128, 512], FP32, tag="ysb")
                        nc.any.tensor_copy(ysb[:], psY[:])
                        nc.sync.dma_start(
                            y_out.rearrange("co b x -> (co b) x")[m * 128:(m + 1) * 128,
                                                                  n2 * 512:(n2 + 1) * 512],
                            ysb[:])

    nc.finalize()
    return nc


_NC_CACHE = None
_LAST_IN_MAPS = None
_LAST_RES = None


def kernel(x, weight, alpha, bias, H_real, H_imag):
    global _NC_CACHE, _LAST_IN_MAPS, _LAST_RES
    x = np.ascontiguousarray(np.asarray(x, np.float32))
    weight = np.ascontiguousarray(np.asarray(weight, np.float32))
    alpha = np.asarray(alpha, np.float32)
    bias = np.asarray(bias, np.float32)
    H_real = np.ascontiguousarray(np.asarray(H_real, np.float32))
    H_imag = np.ascontiguousarray(np.asarray(H_imag, np.float32))

    dft_c, idft, ph = _host_consts()

    # host prep: layout permutations (+ one elementwise +/-), bf16 casts
    w4 = weight.astype(np.float32)
    wm = w4 - w4.transpose(1, 0, 2, 3)            # [i,j,p,q]
    wp = w4 + w4.transpose(1, 0, 2, 3)
    wmT_np = np.ascontiguousarray(
        wm.transpose(2, 3, 1, 0).reshape(9, 65536)).astype(NPBF)
    wpT_np = np.ascontiguousarray(
        wp.transpose(2, 3, 1, 0).reshape(9, 65536)).astype(NPBF)
    wflat_np = np.ascontiguousarray(weight.reshape(128, 4608))
    xt_np = np.ascontiguousarray(
        x.reshape(B * C, PIX).T).astype(NPBF)     # [pix, b*C+c]

    idft_np = np.zeros((9 * 128, PIX), np.float32)
    idft_np[0:2 * F] = idft.astype(np.float32)
    idft_np[2 * F] = 1.0                          # bias row
    idft_np = idft_np.astype(NPBF)

    in_maps = []
    for c in range(N_CORES):
        fsl = slice(FL * c, FL * (c + 1))
        dft_core = np.empty((PIX, F2L), np.float32)
        dft_core[:, 0::2] = dft_c[:, fsl].real
        dft_core[:, 1::2] = dft_c[:, fsl].imag
        ph_core = ph[fsl]                          # [FL, 9]
        in_maps.append({
            "xt_in": xt_np,
            "wflat": wflat_np,
            "wmT": wmT_np,
            "wpT": wpT_np,
            "alpha_in": alpha.reshape(1),
            # [f, k, j, i] -> [f, p, k, jc, i]  (j = jc*128 + p)
            "h2_in": np.ascontiguousarray(np.stack(
                [H_real[fsl].transpose(0, 2, 1),
                 H_imag[fsl].transpose(0, 2, 1)], axis=1)
                .reshape(FL, 2, 2, 128, C)
                .transpose(0, 3, 1, 2, 4)).astype(NPBF),
            "dft_in": dft_core.astype(NPBF),
            "idft_in": idft_np,
            "phre_in": np.ascontiguousarray(ph_core.real.T.astype(np.float32)),
            "nphim_in": np.ascontiguousarray((-ph_core.imag.T).astype(np.float32)),
            "bias_exp": np.repeat(bias[32 * c:32 * (c + 1)], 16).astype(NPBF),
        })

    _LAST_IN_MAPS = in_maps
    if _NC_CACHE is None:
        _NC_CACHE = _build_nc()
    res = bass_utils.run_bass_kernel_spmd(_NC_CACHE, in_maps,
                                          core_ids=list(range(N_CORES)))
    _LAST_RES = res
    y = np.empty((B, C, N, N), np.float32)
    for c in range(N_CORES):
        y[:, 32 * c:32 * (c + 1)] = res.results[c]["y_out"].reshape(
            32, B, N, N).transpose(1, 0, 2, 3)
    return y
